# revision 1
# baseline (speedup 1.0000x reference)
"""ComplexMaxPool2D (K=2) Trainium2 Bass kernel.

Full input x_complex [8, 64, 320, 320] f32. Channels pair up as
(re, im) = (2c, 2c+1); per 2x2 window pick the complex value with max
|z| (argmax, first-wins on sqrt ties) -> output [8, 64, 160, 160].

Sharding: pure data parallel over batch -> core i handles x[i]
([64, 320, 320], 26.2 MB in / 6.55 MB out per core).

Per-core algorithm (memory-bound target = DMA roofline ~91 us/core):
  - View each (re,im) plane pair as 160 row-pairs of 640 f32 (two
    adjacent image rows, contiguous in HBM). One SBUF tile = 128
    partitions x 5 row-pairs = 640 row-pairs = exactly 4 complex-pair
    planes, so each tile loads with ONE DMA per component (re / im)
    and stores with one DMA per component. 8 tiles cover the core.
  - mag = re^2 + im^2: ACT Square ops produce the squares (bit-exact
    f32, HW-verified); the DVE adds them in place.
  - Window argmax by a strictly-greater overwrite chain over the 4
    candidates (a,b,c,d) = (r0k0, r0k1, r1k0, r1k1) using
    copy_predicated; masks are fused scalar_tensor_tensor ops
    computing (m_best * (1+2^-23)) < m_k, which reproduces the
    reference's argmax-over-sqrt(m) tie behavior (sqrt collapses
    ~1-ulp m gaps; verified exact on the seed-0 dataset).
  - Engine split: DVE = mag add + 3 fused compare masks + 2 running
    maxes + 3 predicated overwrites; ACT = squares, base copy, output
    DMAs (emitted one tile late so next-tile squares aren't blocked
    behind them); SYNC = input DMAs. All tiles' squares/adds are half-split (tile 0 quarter-split,
    with FD-chunked DMAs) so the DVE add starts as soon as the first
    square chunks land, shortening fill and per-tile ACT->DVE waits.
    TimelineSim models ~122.5 us/core vs the 91 us DMA roofline
    (32.75 MB traffic at 358 GB/s), DVE 86% occupied. (A TensorE
    fp32 identity-matmul offload of the add was tried and rejected:
    fp32 PE is ~6.5x slower per lane and its latency chain cannot be
    hidden by this scheduler.)
  - This walrus build accepts only ONE sync-wait per instruction, no
    Pool-engine compute, and no custom DVE ops: _split_multi_waits
    hoists extra waits into standalone EventSemaphore instructions.
"""

import sys

for _p in ("/opt/trn_rl_repo",):
    if _p not in sys.path:
        sys.path.insert(0, _p)

import numpy as np

import concourse.bass as bass
import concourse.tile as tile
from concourse import mybir
from concourse import bass_utils

F32 = mybir.dt.float32

# ---- problem constants (hardcoded per contract) ----
B, C2, H, W = 8, 64, 320, 320
NCORES = 8
C = C2 // 2              # 32 complex pairs per core
HO, WO = H // 2, W // 2  # 160 x 160
RP_PER_PART = 5          # row-pairs per SBUF partition
PLANES_PER_TILE = 4      # 128 parts x 5 rp = 640 rp = 4 planes exactly
NTILES = C // PLANES_PER_TILE        # 8
FD_COMP = RP_PER_PART * 2 * W        # 3200 f32 per component per partition
FD_SEL = RP_PER_PART * WO            # 800 selection lanes per partition

# chain comparison epsilon: candidate k overwrites iff m_k > m_best*(1+2^-23)
EPS1 = float(np.float32(1.0 + 2.0 ** -23))


def _split_multi_waits(nc: bass.Bass, max_inline: int = 1) -> None:
    """Hoist all but `max_inline` sync-waits of each instruction into
    standalone EventSemaphore waits on the same engine, placed directly
    before it. The walrus build in this toolchain rejects instructions
    carrying more than one sync-wait command ("Too many sync wait
    commands"); a sequencer executes a standalone wait with identical
    blocking semantics."""
    n = 0
    for f in nc.m.functions:
        for blk in f.blocks:
            out = []
            for inst in blk.instructions:
                si = inst.sync_info
                if si is not None and len(si.on_wait) > max_inline:
                    waits = list(si.on_wait)
                    hoist, keep = waits[:-max_inline], waits[-max_inline:]
                    for w in hoist:
                        out.append(
                            mybir.InstEventSemaphore(
                                name=f"hw{n}_{inst.name}",
                                engine=inst.engine,
                                ins=[],
                                outs=[],
                                sync_info=mybir.SyncInfo(
                                    on_wait=[w], on_update=[]
                                ),
                            )
                        )
                        n += 1
                    inst.sync_info = mybir.SyncInfo(
                        on_wait=keep, on_update=list(si.on_update)
                    )
                out.append(inst)
            blk.instructions = out


def build_program() -> bass.Bass:
    nc = bass.Bass("TRN2", target_bir_lowering=False, debug=False)
    x_dram = nc.dram_tensor("x", [C2 * H * W], F32, kind="ExternalInput")
    o_dram = nc.dram_tensor("out", [C2 * HO * WO], F32, kind="ExternalOutput")

    # per-component views: [pair c][s: re/im][plane elems], channel = 2c+s
    xc = x_dram.ap().rearrange("(c s e) -> c s e", c=C, s=2)
    oc = o_dram.ap().rearrange("(c s e) -> c s e", c=C, s=2)

    with tile.TileContext(nc) as tc:
        with (
            tc.tile_pool(name="xin", bufs=4) as xpool,
            tc.tile_pool(name="sqp", bufs=2) as sqpool,
            tc.tile_pool(name="mgp", bufs=2) as mgpool,
            tc.tile_pool(name="msk", bufs=2) as mpool,
            tc.tile_pool(name="bst", bufs=2) as bpool,
            tc.tile_pool(name="out", bufs=3) as opool,
        ):
            pending_outs = []
            for t in range(NTILES):
                c0 = t * PLANES_PER_TILE

                x_t = xpool.tile([128, 2 * FD_COMP], F32, tag="x", name=f"x{t}")
                ndc = 4 if t == 0 else 1  # tile 0: FD-chunked DMAs for ramp-up
                fstep = FD_COMP // ndc
                for q in range(ndc):
                    for s in (0, 1):  # 0=re plane (ch 2c), 1=im (ch 2c+1)
                        src = xc[c0:c0 + PLANES_PER_TILE, s].rearrange(
                            "c (p f) -> c p f", f=FD_COMP
                        )[:, :, q * fstep:(q + 1) * fstep]
                        nc.sync.dma_start(
                            x_t[:, s * FD_COMP + q * fstep:
                                s * FD_COMP + (q + 1) * fstep], src
                        )

                # views: x[p, s, u, r, j, k]
                xv = x_t[:].rearrange(
                    "p (s u r j k) -> p s u r j k",
                    s=2, u=RP_PER_PART, r=2, j=WO, k=2,
                )

                def x_cand(r, k):
                    return xv[:, :, :, r, :, k]

                # squares on ACT (bit-exact f32); sq_re goes straight into
                # the mag tile, then the DVE adds sq_im in place.
                mag_t = mgpool.tile([128, FD_COMP], F32, tag="mag",
                                    name=f"mag{t}")
                sq_im = sqpool.tile([128, FD_COMP], F32, tag="sqim",
                                    name=f"sqim{t}")
                nq = 4 if t == 0 else 2  # split for overlap
                step = FD_COMP // nq
                for q in range(nq):
                    nc.scalar.square(mag_t[:, q * step:(q + 1) * step],
                                     x_t[:, q * step:(q + 1) * step])
                    nc.scalar.square(sq_im[:, q * step:(q + 1) * step],
                                     x_t[:, FD_COMP + q * step:
                                         FD_COMP + (q + 1) * step])
                out_t = opool.tile([128, 2 * FD_SEL], F32, tag="o", name=f"o{t}")
                ov = out_t[:].rearrange(
                    "p (s u j) -> p s u j", s=2, u=RP_PER_PART, j=WO
                )
                # base: candidate a = (r0, k0) — before the deferred outs so
                # ACT isn't stuck behind their DVE-chain waits.
                nc.scalar.copy(ov, x_cand(0, 0))
                if pending_outs:
                    pending_outs.pop(0)()
                for q in range(nq):
                    lo, hi = q * step, (q + 1) * step
                    nc.vector.tensor_tensor(
                        mag_t[:, lo:hi], mag_t[:, lo:hi], sq_im[:, lo:hi],
                        mybir.AluOpType.add,
                    )

                mv = mag_t[:].rearrange(
                    "p (u r j k) -> p u r j k", u=RP_PER_PART, r=2, j=WO, k=2
                )

                def m_cand(r, k):
                    return mv[:, :, r, :, k]

                def as3(ap):  # [128, 800] -> [128, 5, 160]
                    return ap.rearrange("p (u j) -> p u j", u=RP_PER_PART)

                def bcast(ap):  # [128, 800] -> [128, 2, 5, 160] (step-0 s dim)
                    # bitcast f32 {1.0, 0.0} -> int32 {0x3F800000, 0}: BIR
                    # requires an integer mask dtype; nonzero means true.
                    return (
                        as3(ap).unsqueeze(1)
                        .broadcast_to((128, 2, RP_PER_PART, WO))
                        .bitcast(mybir.dt.int32)
                    )

                # chain step b = (r0, k1): b wins iff EPS1*m_a < m_b
                mask1 = mpool.tile([128, FD_SEL], F32, tag="m1", name=f"mask1_{t}")
                nc.vector.scalar_tensor_tensor(
                    as3(mask1[:]), m_cand(0, 0), EPS1, m_cand(0, 1),
                    op0=mybir.AluOpType.mult, op1=mybir.AluOpType.is_lt,
                )
                best1 = bpool.tile([128, FD_SEL], F32, tag="b1", name=f"best1_{t}")
                nc.vector.tensor_tensor(
                    as3(best1[:]), m_cand(0, 0), m_cand(0, 1),
                    mybir.AluOpType.max,
                )
                nc.vector.copy_predicated(ov, bcast(mask1[:]), x_cand(0, 1))

                # chain step c = (r1, k0)
                mask2 = mpool.tile([128, FD_SEL], F32, tag="m2", name=f"mask2_{t}")
                nc.vector.scalar_tensor_tensor(
                    as3(mask2[:]), as3(best1[:]), EPS1, m_cand(1, 0),
                    op0=mybir.AluOpType.mult, op1=mybir.AluOpType.is_lt,
                )
                best2 = bpool.tile([128, FD_SEL], F32, tag="b2", name=f"best2_{t}")
                nc.vector.tensor_tensor(
                    as3(best2[:]), as3(best1[:]), m_cand(1, 0),
                    mybir.AluOpType.max,
                )
                nc.vector.copy_predicated(ov, bcast(mask2[:]), x_cand(1, 0))

                # chain step d = (r1, k1)
                mask3 = mpool.tile([128, FD_SEL], F32, tag="m3", name=f"mask3_{t}")
                nc.vector.scalar_tensor_tensor(
                    as3(mask3[:]), as3(best2[:]), EPS1, m_cand(1, 1),
                    op0=mybir.AluOpType.mult, op1=mybir.AluOpType.is_lt,
                )
                nc.vector.copy_predicated(ov, bcast(mask3[:]), x_cand(1, 1))

                # output DMAs ride the ACT ring, but are EMITTED one tile
                # late (after tile t+1's squares) so the ACT sequencer runs
                # next squares before blocking on this tile's chain.
                def emit_outs(c0=c0, out_t=out_t):
                    for s in (0, 1):
                        dst = oc[c0:c0 + PLANES_PER_TILE, s].rearrange(
                            "c (p f) -> c p f", f=FD_SEL
                        )
                        nc.scalar.dma_start(
                            dst, out_t[:, s * FD_SEL:(s + 1) * FD_SEL]
                        )
                pending_outs.append(emit_outs)

            for f_ in pending_outs:
                f_()

    mybir.codegen_inst_isa_subclasses(nc)
    _split_multi_waits(nc)
    return nc


_NC = None
LAST_RESULT = None


def _get_nc() -> bass.Bass:
    global _NC
    if _NC is None:
        _NC = build_program()
    return _NC


def kernel(x_complex: np.ndarray) -> np.ndarray:
    assert x_complex.shape == (B, C2, H, W), x_complex.shape
    x = np.ascontiguousarray(x_complex, dtype=np.float32)
    nc = _get_nc()
    in_maps = [{"x": x[i].reshape(-1)} for i in range(NCORES)]
    global LAST_RESULT, _NC
    try:
        LAST_RESULT = bass_utils.run_bass_kernel_spmd(
            nc, in_maps, core_ids=list(range(NCORES))
        )
    except Exception:
        # The axon terminal can refuse re-executing a cached executable
        # (repeat kernel() calls in one process). A freshly built program
        # yields a new executable; the NEFF compile itself is disk-cached.
        _NC = None
        LAST_RESULT = bass_utils.run_bass_kernel_spmd(
            _get_nc(), in_maps, core_ids=list(range(NCORES))
        )
    out = np.stack(
        [LAST_RESULT.results[i]["out"].reshape(C2, HO, WO) for i in range(NCORES)],
        axis=0,
    )
    return out



# revision 6
# speedup vs baseline: 1.0662x; 1.0662x over previous
"""ComplexMaxPool2D (K=2) Trainium2 Bass kernel.

Full input x_complex [8, 64, 320, 320] f32. Channels pair up as
(re, im) = (2c, 2c+1); per 2x2 window pick the complex value with max
|z| (argmax, first-wins on sqrt ties) -> output [8, 64, 160, 160].

Sharding: pure data parallel over batch -> core i handles x[i]
([64, 320, 320], 26.2 MB in / 6.55 MB out per core).

Per-core algorithm (memory-bound target = DMA roofline ~91 us/core):
  - View each (re,im) plane pair as 160 row-pairs of 640 f32 (two
    adjacent image rows, contiguous in HBM). One SBUF tile = 128
    partitions x 5 row-pairs = 640 row-pairs = exactly 4 complex-pair
    planes, so each tile loads with ONE DMA per component (re / im)
    and stores with one DMA per component. 8 tiles cover the core.
  - mag = re^2 + im^2: ACT Square ops produce the squares (bit-exact
    f32, HW-verified); the DVE adds them in place.
  - Window argmax by a strictly-greater overwrite chain over the 4
    candidates (a,b,c,d) = (r0k0, r0k1, r1k0, r1k1) using
    copy_predicated; masks are fused scalar_tensor_tensor ops
    computing (m_best * (1+2^-23)) < m_k, which reproduces the
    reference's argmax-over-sqrt(m) tie behavior (sqrt collapses
    ~1-ulp m gaps; verified exact on the seed-0 dataset).
  - Engine split: DVE = mag add + 3 fused compare masks + 2 running
    maxes + 3 predicated overwrites; ACT = squares, base copy, output
    DMAs (emitted one tile late so next-tile squares aren't blocked
    behind them); SYNC = input DMAs. All tiles' squares/adds are half-split (tile 0 quarter-split,
    with FD-chunked DMAs) so the DVE add starts as soon as the first
    square chunks land, shortening fill and per-tile ACT->DVE waits.
    TimelineSim models ~122.5 us/core vs the 91 us DMA roofline
    (32.75 MB traffic at 358 GB/s), DVE 86% occupied. (A TensorE
    fp32 identity-matmul offload of the add was tried and rejected:
    fp32 PE is ~6.5x slower per lane and its latency chain cannot be
    hidden by this scheduler.)
  - This walrus build accepts only ONE sync-wait per instruction, no
    Pool-engine compute, and no custom DVE ops: _split_multi_waits
    hoists extra waits into standalone EventSemaphore instructions.
"""

import sys

for _p in ("/opt/trn_rl_repo",):
    if _p not in sys.path:
        sys.path.insert(0, _p)

import numpy as np

import concourse.bass as bass
import concourse.tile as tile
from concourse import mybir
from concourse import bass_utils

F32 = mybir.dt.float32

# ---- problem constants (hardcoded per contract) ----
B, C2, H, W = 8, 64, 320, 320
NCORES = 8
C = C2 // 2              # 32 complex pairs per core
HO, WO = H // 2, W // 2  # 160 x 160
RP_PER_PART = 5          # row-pairs per SBUF partition
PLANES_PER_TILE = 4      # 128 parts x 5 rp = 640 rp = 4 planes exactly
NTILES = C // PLANES_PER_TILE        # 8
FD_COMP = RP_PER_PART * 2 * W        # 3200 f32 per component per partition
FD_SEL = RP_PER_PART * WO            # 800 selection lanes per partition

# chain comparison epsilon: candidate k overwrites iff m_k > m_best*(1+2^-23)
EPS1 = float(np.float32(1.0 + 2.0 ** -23))


def _split_multi_waits(nc: bass.Bass, max_inline: int = 1) -> None:
    """Hoist all but `max_inline` sync-waits of each instruction into
    standalone EventSemaphore waits on the same engine, placed directly
    before it. The walrus build in this toolchain rejects instructions
    carrying more than one sync-wait command ("Too many sync wait
    commands"); a sequencer executes a standalone wait with identical
    blocking semantics."""
    n = 0
    for f in nc.m.functions:
        for blk in f.blocks:
            out = []
            for inst in blk.instructions:
                si = inst.sync_info
                if si is not None and len(si.on_wait) > max_inline:
                    waits = list(si.on_wait)
                    hoist, keep = waits[:-max_inline], waits[-max_inline:]
                    for w in hoist:
                        out.append(
                            mybir.InstEventSemaphore(
                                name=f"hw{n}_{inst.name}",
                                engine=inst.engine,
                                ins=[],
                                outs=[],
                                sync_info=mybir.SyncInfo(
                                    on_wait=[w], on_update=[]
                                ),
                            )
                        )
                        n += 1
                    inst.sync_info = mybir.SyncInfo(
                        on_wait=keep, on_update=list(si.on_update)
                    )
                out.append(inst)
            blk.instructions = out


def build_program() -> bass.Bass:
    nc = bass.Bass("TRN2", target_bir_lowering=False, debug=False)
    x_dram = nc.dram_tensor("x", [C2 * H * W], F32, kind="ExternalInput")
    o_dram = nc.dram_tensor("out", [C2 * HO * WO], F32, kind="ExternalOutput")

    # per-component views: [pair c][s: re/im][plane elems], channel = 2c+s
    xc = x_dram.ap().rearrange("(c s e) -> c s e", c=C, s=2)
    oc = o_dram.ap().rearrange("(c s e) -> c s e", c=C, s=2)

    with tile.TileContext(nc) as tc:
        with (
            tc.tile_pool(name="xin", bufs=4) as xpool,
            tc.tile_pool(name="sqp", bufs=2) as sqpool,
            tc.tile_pool(name="mgp", bufs=2) as mgpool,
            tc.tile_pool(name="msk", bufs=2) as mpool,
            tc.tile_pool(name="bst", bufs=2) as bpool,
            tc.tile_pool(name="out", bufs=3) as opool,
        ):
            pending_outs = []
            for t in range(NTILES):
                c0 = t * PLANES_PER_TILE

                x_t = xpool.tile([128, 2 * FD_COMP], F32, tag="x", name=f"x{t}")
                ndc = 4 if t == 0 else 1  # tile 0: FD-chunked DMAs for ramp-up
                fstep = FD_COMP // ndc
                for q in range(ndc):
                    for s in (0, 1):  # 0=re plane (ch 2c), 1=im (ch 2c+1)
                        src = xc[c0:c0 + PLANES_PER_TILE, s].rearrange(
                            "c (p f) -> c p f", f=FD_COMP
                        )[:, :, q * fstep:(q + 1) * fstep]
                        nc.sync.dma_start(
                            x_t[:, s * FD_COMP + q * fstep:
                                s * FD_COMP + (q + 1) * fstep], src
                        )

                # views: x[p, s, u, r, j, k]
                xv = x_t[:].rearrange(
                    "p (s u r j k) -> p s u r j k",
                    s=2, u=RP_PER_PART, r=2, j=WO, k=2,
                )

                def x_cand(r, k):
                    return xv[:, :, :, r, :, k]

                # squares on ACT (bit-exact f32); sq_re goes straight into
                # the mag tile, then the DVE adds sq_im in place.
                mag_t = mgpool.tile([128, FD_COMP], F32, tag="mag",
                                    name=f"mag{t}")
                sq_im = sqpool.tile([128, FD_COMP], F32, tag="sqim",
                                    name=f"sqim{t}")
                nq = 4 if t == 0 else 2  # split for overlap
                step = FD_COMP // nq
                for q in range(nq):
                    nc.scalar.square(mag_t[:, q * step:(q + 1) * step],
                                     x_t[:, q * step:(q + 1) * step])
                    nc.scalar.square(sq_im[:, q * step:(q + 1) * step],
                                     x_t[:, FD_COMP + q * step:
                                         FD_COMP + (q + 1) * step])
                out_t = opool.tile([128, 2 * FD_SEL], F32, tag="o", name=f"o{t}")
                ov = out_t[:].rearrange(
                    "p (s u j) -> p s u j", s=2, u=RP_PER_PART, j=WO
                )
                # base: candidate a = (r0, k0) — before the deferred outs so
                # ACT isn't stuck behind their DVE-chain waits.
                nc.scalar.copy(ov, x_cand(0, 0))
                if pending_outs:
                    pending_outs.pop(0)()
                # mag add on the Pool engine (gpsimd): frees ~3.3us/tile of
                # DVE time; Pool is otherwise idle. (Pool TensorTensor works
                # in this walrus build once multi-waits are hoisted.)
                for q in range(nq):
                    lo, hi = q * step, (q + 1) * step
                    nc.gpsimd.tensor_tensor(
                        mag_t[:, lo:hi], mag_t[:, lo:hi], sq_im[:, lo:hi],
                        mybir.AluOpType.add,
                    )

                mv = mag_t[:].rearrange(
                    "p (u r j k) -> p u r j k", u=RP_PER_PART, r=2, j=WO, k=2
                )

                def m_cand(r, k):
                    return mv[:, :, r, :, k]

                def as3(ap):  # [128, 800] -> [128, 5, 160]
                    return ap.rearrange("p (u j) -> p u j", u=RP_PER_PART)

                def bcast(ap):  # [128, 800] -> [128, 2, 5, 160] (step-0 s dim)
                    # bitcast f32 {1.0, 0.0} -> int32 {0x3F800000, 0}: BIR
                    # requires an integer mask dtype; nonzero means true.
                    return (
                        as3(ap).unsqueeze(1)
                        .broadcast_to((128, 2, RP_PER_PART, WO))
                        .bitcast(mybir.dt.int32)
                    )

                # chain step b = (r0, k1): b wins iff EPS1*m_a < m_b
                mask1 = mpool.tile([128, FD_SEL], F32, tag="m1", name=f"mask1_{t}")
                nc.vector.scalar_tensor_tensor(
                    as3(mask1[:]), m_cand(0, 0), EPS1, m_cand(0, 1),
                    op0=mybir.AluOpType.mult, op1=mybir.AluOpType.is_lt,
                )
                best1 = bpool.tile([128, FD_SEL], F32, tag="b1", name=f"best1_{t}")
                nc.vector.tensor_tensor(
                    as3(best1[:]), m_cand(0, 0), m_cand(0, 1),
                    mybir.AluOpType.max,
                )
                nc.vector.copy_predicated(ov, bcast(mask1[:]), x_cand(0, 1))

                # chain step c = (r1, k0)
                mask2 = mpool.tile([128, FD_SEL], F32, tag="m2", name=f"mask2_{t}")
                nc.vector.scalar_tensor_tensor(
                    as3(mask2[:]), as3(best1[:]), EPS1, m_cand(1, 0),
                    op0=mybir.AluOpType.mult, op1=mybir.AluOpType.is_lt,
                )
                best2 = bpool.tile([128, FD_SEL], F32, tag="b2", name=f"best2_{t}")
                nc.vector.tensor_tensor(
                    as3(best2[:]), as3(best1[:]), m_cand(1, 0),
                    mybir.AluOpType.max,
                )
                nc.vector.copy_predicated(ov, bcast(mask2[:]), x_cand(1, 0))

                # chain step d = (r1, k1)
                mask3 = mpool.tile([128, FD_SEL], F32, tag="m3", name=f"mask3_{t}")
                nc.vector.scalar_tensor_tensor(
                    as3(mask3[:]), as3(best2[:]), EPS1, m_cand(1, 1),
                    op0=mybir.AluOpType.mult, op1=mybir.AluOpType.is_lt,
                )
                nc.vector.copy_predicated(ov, bcast(mask3[:]), x_cand(1, 1))

                # output DMAs ride the ACT ring, but are EMITTED one tile
                # late (after tile t+1's squares) so the ACT sequencer runs
                # next squares before blocking on this tile's chain.
                def emit_outs(c0=c0, out_t=out_t):
                    for s in (0, 1):
                        dst = oc[c0:c0 + PLANES_PER_TILE, s].rearrange(
                            "c (p f) -> c p f", f=FD_SEL
                        )
                        nc.scalar.dma_start(
                            dst, out_t[:, s * FD_SEL:(s + 1) * FD_SEL]
                        )
                pending_outs.append(emit_outs)

            for f_ in pending_outs:
                f_()

    mybir.codegen_inst_isa_subclasses(nc)
    _split_multi_waits(nc)
    return nc


_NC = None
LAST_RESULT = None


def _get_nc() -> bass.Bass:
    global _NC
    if _NC is None:
        _NC = build_program()
    return _NC


def kernel(x_complex: np.ndarray) -> np.ndarray:
    assert x_complex.shape == (B, C2, H, W), x_complex.shape
    x = np.ascontiguousarray(x_complex, dtype=np.float32)
    nc = _get_nc()
    in_maps = [{"x": x[i].reshape(-1)} for i in range(NCORES)]
    global LAST_RESULT, _NC
    try:
        LAST_RESULT = bass_utils.run_bass_kernel_spmd(
            nc, in_maps, core_ids=list(range(NCORES))
        )
    except Exception:
        # The axon terminal can refuse re-executing a cached executable
        # (repeat kernel() calls in one process). A freshly built program
        # yields a new executable; the NEFF compile itself is disk-cached.
        _NC = None
        LAST_RESULT = bass_utils.run_bass_kernel_spmd(
            _get_nc(), in_maps, core_ids=list(range(NCORES))
        )
    out = np.stack(
        [LAST_RESULT.results[i]["out"].reshape(C2, HO, WO) for i in range(NCORES)],
        axis=0,
    )
    return out



# revision 14
# speedup vs baseline: 1.1784x; 1.1052x over previous
"""ComplexMaxPool2D (K=2) Trainium2 Bass kernel.

Full input x_complex [8, 64, 320, 320] f32. Channels pair up as
(re, im) = (2c, 2c+1); per 2x2 window pick the complex value with max
|z| (argmax, first-wins on sqrt ties) -> output [8, 64, 160, 160].

Sharding: pure data parallel over batch -> core i handles x[i]
([64, 320, 320], 26.2 MB in / 6.55 MB out per core).

Per-core algorithm (memory-bound target = DMA roofline ~91 us/core):
  - View each (re,im) plane pair as 160 row-pairs of 640 f32 (two
    adjacent image rows, contiguous in HBM). One SBUF tile = 128
    partitions x 5 row-pairs = 640 row-pairs = exactly 4 complex-pair
    planes, so each tile loads with ONE DMA per component (re / im)
    and stores with one DMA per component. 8 tiles cover the core.
  - mag = re^2 + im^2: ACT Square ops produce the squares (bit-exact
    f32, HW-verified); the DVE adds them in place.
  - Window argmax by a strictly-greater overwrite chain over the 4
    candidates (a,b,c,d) = (r0k0, r0k1, r1k0, r1k1) using
    copy_predicated; masks are fused scalar_tensor_tensor ops
    computing (m_best * (1+2^-23)) < m_k, which reproduces the
    reference's argmax-over-sqrt(m) tie behavior (sqrt collapses
    ~1-ulp m gaps; verified exact on the seed-0 dataset).
  - Engine split: DVE = mag add + 3 fused compare masks + 2 running
    maxes + 3 predicated overwrites; ACT = squares, base copy, output
    DMAs (emitted one tile late so next-tile squares aren't blocked
    behind them); SYNC = input DMAs. All tiles' squares/adds are half-split (tile 0 quarter-split,
    with FD-chunked DMAs) so the DVE add starts as soon as the first
    square chunks land, shortening fill and per-tile ACT->DVE waits.
    TimelineSim models ~122.5 us/core vs the 91 us DMA roofline
    (32.75 MB traffic at 358 GB/s), DVE 86% occupied. (A TensorE
    fp32 identity-matmul offload of the add was tried and rejected:
    fp32 PE is ~6.5x slower per lane and its latency chain cannot be
    hidden by this scheduler.)
  - This walrus build accepts only ONE sync-wait per instruction, no
    Pool-engine compute, and no custom DVE ops: _split_multi_waits
    hoists extra waits into standalone EventSemaphore instructions.
"""

import sys

for _p in ("/opt/trn_rl_repo",):
    if _p not in sys.path:
        sys.path.insert(0, _p)

import numpy as np

import concourse.bass as bass
import concourse.tile as tile
from concourse import mybir
from concourse import bass_utils

F32 = mybir.dt.float32

# ---- problem constants (hardcoded per contract) ----
B, C2, H, W = 8, 64, 320, 320
NCORES = 8
C = C2 // 2              # 32 complex pairs per core
HO, WO = H // 2, W // 2  # 160 x 160
RP_PER_PART = 5          # row-pairs per SBUF partition
PLANES_PER_TILE = 4      # 128 parts x 5 rp = 640 rp = 4 planes exactly
NTILES = C // PLANES_PER_TILE        # 8
FD_COMP = RP_PER_PART * 2 * W        # 3200 f32 per component per partition
FD_SEL = RP_PER_PART * WO            # 800 selection lanes per partition

# chain comparison epsilon: candidate k overwrites iff m_k > m_best*(1+2^-23)
EPS1 = float(np.float32(1.0 + 2.0 ** -23))


def _split_multi_waits(nc: bass.Bass, max_inline: int = 1) -> None:
    """Hoist all but `max_inline` sync-waits of each instruction into
    standalone EventSemaphore waits on the same engine, placed directly
    before it. The walrus build in this toolchain rejects instructions
    carrying more than one sync-wait command ("Too many sync wait
    commands"); a sequencer executes a standalone wait with identical
    blocking semantics."""
    n = 0
    for f in nc.m.functions:
        for blk in f.blocks:
            out = []
            for inst in blk.instructions:
                si = inst.sync_info
                if si is not None and len(si.on_wait) > max_inline:
                    waits = list(si.on_wait)
                    hoist, keep = waits[:-max_inline], waits[-max_inline:]
                    for w in hoist:
                        out.append(
                            mybir.InstEventSemaphore(
                                name=f"hw{n}_{inst.name}",
                                engine=inst.engine,
                                ins=[],
                                outs=[],
                                sync_info=mybir.SyncInfo(
                                    on_wait=[w], on_update=[]
                                ),
                            )
                        )
                        n += 1
                    inst.sync_info = mybir.SyncInfo(
                        on_wait=keep, on_update=list(si.on_update)
                    )
                out.append(inst)
            blk.instructions = out


def build_program() -> bass.Bass:
    nc = bass.Bass("TRN2", target_bir_lowering=False, debug=False)
    x_dram = nc.dram_tensor("x", [C2 * H * W], F32, kind="ExternalInput")
    o_dram = nc.dram_tensor("out", [C2 * HO * WO], F32, kind="ExternalOutput")

    # per-component views: [pair c][s: re/im][plane elems], channel = 2c+s
    xc = x_dram.ap().rearrange("(c s e) -> c s e", c=C, s=2)
    oc = o_dram.ap().rearrange("(c s e) -> c s e", c=C, s=2)

    with tile.TileContext(nc) as tc:
        with (
            tc.tile_pool(name="xin", bufs=3) as xpool,
            tc.tile_pool(name="sqp", bufs=2) as sqpool,
            tc.tile_pool(name="mgp", bufs=2) as mgpool,
            tc.tile_pool(name="msk", bufs=2) as mpool,
            tc.tile_pool(name="bst", bufs=2) as bpool,
            # all 8 out tiles stay live: their DMAs are issued only after the
            # full input stream so the DMA engines never starve mid-stream.
            tc.tile_pool(name="out", bufs=NTILES) as opool,
        ):
            pending_outs = []
            for t in range(NTILES):
                c0 = t * PLANES_PER_TILE

                x_t = xpool.tile([128, 2 * FD_COMP], F32, tag="x", name=f"x{t}")
                ndc = 4 if t == 0 else 1  # tile 0: FD-chunked DMAs for ramp-up
                fstep = FD_COMP // ndc
                for q in range(ndc):
                    for s in (0, 1):  # 0=re plane (ch 2c), 1=im (ch 2c+1)
                        src = xc[c0:c0 + PLANES_PER_TILE, s].rearrange(
                            "c (p f) -> c p f", f=FD_COMP
                        )[:, :, q * fstep:(q + 1) * fstep]
                        nc.sync.dma_start(
                            x_t[:, s * FD_COMP + q * fstep:
                                s * FD_COMP + (q + 1) * fstep], src
                        )

                # views: x[p, s, u, r, j, k]
                xv = x_t[:].rearrange(
                    "p (s u r j k) -> p s u r j k",
                    s=2, u=RP_PER_PART, r=2, j=WO, k=2,
                )

                def x_cand(r, k):
                    return xv[:, :, :, r, :, k]

                # squares on ACT (bit-exact f32); sq_re goes straight into
                # the mag tile, then the DVE adds sq_im in place.
                mag_t = mgpool.tile([128, FD_COMP], F32, tag="mag",
                                    name=f"mag{t}")
                sq_im = sqpool.tile([128, FD_COMP], F32, tag="sqim",
                                    name=f"sqim{t}")
                nq = 4 if t == 0 else 2  # split for overlap
                step = FD_COMP // nq
                for q in range(nq):
                    nc.scalar.square(mag_t[:, q * step:(q + 1) * step],
                                     x_t[:, q * step:(q + 1) * step])
                    nc.scalar.square(sq_im[:, q * step:(q + 1) * step],
                                     x_t[:, FD_COMP + q * step:
                                         FD_COMP + (q + 1) * step])
                out_t = opool.tile([128, 2 * FD_SEL], F32, tag="o", name=f"o{t}")
                ov = out_t[:].rearrange(
                    "p (s u j) -> p s u j", s=2, u=RP_PER_PART, j=WO
                )
                # base: candidate a = (r0, k0) — before the deferred outs so
                # ACT isn't stuck behind their DVE-chain waits.
                nc.scalar.copy(ov, x_cand(0, 0))
                # mag add on the Pool engine (gpsimd): frees ~3.3us/tile of
                # DVE time; Pool is otherwise idle. (Pool TensorTensor works
                # in this walrus build once multi-waits are hoisted.)
                for q in range(nq):
                    lo, hi = q * step, (q + 1) * step
                    nc.gpsimd.tensor_tensor(
                        mag_t[:, lo:hi], mag_t[:, lo:hi], sq_im[:, lo:hi],
                        mybir.AluOpType.add,
                    )

                mv = mag_t[:].rearrange(
                    "p (u r j k) -> p u r j k", u=RP_PER_PART, r=2, j=WO, k=2
                )

                def m_cand(r, k):
                    return mv[:, :, r, :, k]

                def as3(ap):  # [128, 800] -> [128, 5, 160]
                    return ap.rearrange("p (u j) -> p u j", u=RP_PER_PART)

                def bcast(ap):  # [128, 800] -> [128, 2, 5, 160] (step-0 s dim)
                    # bitcast f32 {1.0, 0.0} -> int32 {0x3F800000, 0}: BIR
                    # requires an integer mask dtype; nonzero means true.
                    return (
                        as3(ap).unsqueeze(1)
                        .broadcast_to((128, 2, RP_PER_PART, WO))
                        .bitcast(mybir.dt.int32)
                    )

                # chain step b = (r0, k1): b wins iff EPS1*m_a < m_b
                mask1 = mpool.tile([128, FD_SEL], F32, tag="m", name=f"mask1_{t}")
                nc.vector.scalar_tensor_tensor(
                    as3(mask1[:]), m_cand(0, 0), EPS1, m_cand(0, 1),
                    op0=mybir.AluOpType.mult, op1=mybir.AluOpType.is_lt,
                )
                best1 = bpool.tile([128, FD_SEL], F32, tag="b", name=f"best1_{t}")
                nc.vector.tensor_tensor(
                    as3(best1[:]), m_cand(0, 0), m_cand(0, 1),
                    mybir.AluOpType.max,
                )
                nc.vector.copy_predicated(ov, bcast(mask1[:]), x_cand(0, 1))

                # chain step c = (r1, k0)
                mask2 = mpool.tile([128, FD_SEL], F32, tag="m", name=f"mask2_{t}")
                nc.vector.scalar_tensor_tensor(
                    as3(mask2[:]), as3(best1[:]), EPS1, m_cand(1, 0),
                    op0=mybir.AluOpType.mult, op1=mybir.AluOpType.is_lt,
                )
                best2 = bpool.tile([128, FD_SEL], F32, tag="b", name=f"best2_{t}")
                nc.vector.tensor_tensor(
                    as3(best2[:]), as3(best1[:]), m_cand(1, 0),
                    mybir.AluOpType.max,
                )
                nc.vector.copy_predicated(ov, bcast(mask2[:]), x_cand(1, 0))

                # chain step d = (r1, k1)
                mask3 = mpool.tile([128, FD_SEL], F32, tag="m", name=f"mask3_{t}")
                nc.vector.scalar_tensor_tensor(
                    as3(mask3[:]), as3(best2[:]), EPS1, m_cand(1, 1),
                    op0=mybir.AluOpType.mult, op1=mybir.AluOpType.is_lt,
                )
                nc.vector.copy_predicated(ov, bcast(mask3[:]), x_cand(1, 1))

                # output DMAs are deferred: queued after the full input
                # stream (see below).
                pending_outs.append((c0, out_t))

            # All output DMAs ride the SP ring AFTER the 16 input DMAs: the
            # DMA engines chew through inputs at full rate (~73us), then the
            # accumulated output tiles stream out back-to-back (~18us) with
            # no mid-stream starvation.
            for c0, out_t in pending_outs:
                for s in (0, 1):
                    dst = oc[c0:c0 + PLANES_PER_TILE, s].rearrange(
                        "c (p f) -> c p f", f=FD_SEL
                    )
                    nc.sync.dma_start(
                        dst, out_t[:, s * FD_SEL:(s + 1) * FD_SEL]
                    )

    mybir.codegen_inst_isa_subclasses(nc)
    _split_multi_waits(nc)
    return nc


_NC = None
LAST_RESULT = None


def _get_nc() -> bass.Bass:
    global _NC
    if _NC is None:
        _NC = build_program()
    return _NC


def kernel(x_complex: np.ndarray) -> np.ndarray:
    assert x_complex.shape == (B, C2, H, W), x_complex.shape
    x = np.ascontiguousarray(x_complex, dtype=np.float32)
    nc = _get_nc()
    in_maps = [{"x": x[i].reshape(-1)} for i in range(NCORES)]
    global LAST_RESULT, _NC
    try:
        LAST_RESULT = bass_utils.run_bass_kernel_spmd(
            nc, in_maps, core_ids=list(range(NCORES))
        )
    except Exception:
        # The axon terminal can refuse re-executing a cached executable
        # (repeat kernel() calls in one process). A freshly built program
        # yields a new executable; the NEFF compile itself is disk-cached.
        _NC = None
        LAST_RESULT = bass_utils.run_bass_kernel_spmd(
            _get_nc(), in_maps, core_ids=list(range(NCORES))
        )
    out = np.stack(
        [LAST_RESULT.results[i]["out"].reshape(C2, HO, WO) for i in range(NCORES)],
        axis=0,
    )
    return out

